# revision 18
# baseline (speedup 1.0000x reference)
"""Single-head attention (B=8, T=4096, E=768, H=64) on 8 TRN2 NeuronCores.

Sharding: data-parallel over batch B — one batch element per core, Q/K/V
projection weights replicated.

v2 design: wave-interleaved setup + attention so the Activation engine
(the exp bottleneck, ~137us of work at 1024-wide tiles) starts at ~8us
instead of after the full setup phase.

Per core:
  consts: identity, merged W_qk [128e, 64q|64k] chunks, W_v chunks,
          per-partition scale vector (SCALE on rows 0-63, 1.0 on 64-127)
          and bias vector (SCALE*bq rows 0-63, bk rows 64-127); ones row +
          bv row for the v-bias rank-1 matmul.

  4 waves over t (1024 rows each); wave w emits:
    - 2 SWDGE cast-load DMAs (f32->bf16) of 4 t-tiles each  [Pool queue]
    - PE transposes x 128x128 blocks -> xT slices (6 per t-tile batched in
      one PSUM tile, drained by Pool tensor_copy)
    - merged qk projection (6 matmuls per 512-chunk, W stationary) + ACT
      Identity with per-partition scale/bias -> qkT [128, T] (q rows 0-63
      pre-scaled by 1/sqrt(H), k rows 64-127)
    - v tiles direct: psv[128s, 64h] = sum_c xT_c^T @ Wv_c + ones x bv
      (xT stationary), Pool-copied into v1 [128, s, 65] (col 64 = 1.0)
    - attention iterations (g=0, s in wave) interleaved right away

  attention (software-pipelined, q-groups of GQ=1024):
    S^T block [128s, gq] = kT_s.T @ qT on PE (2 x 512-wide matmuls into
    one 2-bank PSUM tile); ONE exp over [128, 1024] on ACT (wider tiles
    amortize the ~185ns SBUF-access setup per instruction); out^T
    [65, gq] += [v|1]_s.T @ exp accumulated in PSUM (row 64 = softmax
    denominator). MM1 of iter i+1 is emitted before MM2 of iter i.
    tail: PE-transpose out^T blocks, multiply by reciprocal of the
    denominator, DMA [t, h] blocks to DRAM on the SP queue.

  All matmul-facing tensors are bf16 (measured rel err 2.6e-3 vs the fp32
  reference; gate is 2e-2).
"""

import os
import sys

for _p in ("/opt/trn_rl_repo", "/root/.axon_site/_ro/trn_rl_repo"):
    if os.path.isdir(_p) and _p not in sys.path:
        sys.path.insert(0, _p)

import numpy as np

import concourse.bacc as bacc
import concourse.tile as tile
from concourse import mybir
from concourse.bass_utils import run_bass_kernel_spmd
from concourse.masks import make_identity

B, T, E, H = 8, 4096, 768, 64
P = 128
NE = E // P            # 6 e-chunks
NT = T // P            # 32 t/s tiles
SCALE = float(H) ** -0.5

F32 = mybir.dt.float32
BF16 = mybir.dt.bfloat16

WAVE_T = 1024          # t rows per setup wave
NW = T // WAVE_T       # 4 waves
WT = WAVE_T // P       # 8 t-tiles per wave


def build_nc(attn_dtype=BF16, proj_dtype=BF16, reps=1, rep_scope="all"):
    del attn_dtype, proj_dtype  # bf16-only in v2; args kept for the harness
    nc = bacc.Bacc("TRN2", target_bir_lowering=False, debug=False, num_devices=8)

    x = nc.dram_tensor("x", [T, E], F32, kind="ExternalInput")
    wq = nc.dram_tensor("Wq", [E, H], F32, kind="ExternalInput")
    wk = nc.dram_tensor("Wk", [E, H], F32, kind="ExternalInput")
    wv = nc.dram_tensor("Wv", [E, H], F32, kind="ExternalInput")
    bq = nc.dram_tensor("bq", [H], F32, kind="ExternalInput")
    bk = nc.dram_tensor("bk", [H], F32, kind="ExternalInput")
    bv = nc.dram_tensor("bv", [H], F32, kind="ExternalInput")
    out = nc.dram_tensor("out", [T, H], F32, kind="ExternalOutput")

    with tile.TileContext(nc) as tc:
        with tc.tile_pool(name="consts", bufs=1) as consts:
            ident = consts.tile([P, P], F32)
            make_identity(nc, ident)
            identb = consts.tile([P, P], BF16, tag="identb")
            nc.vector.tensor_copy(identb, ident)

            w_tiles = {}
            for name, wdram in (("q", wq), ("k", wk), ("v", wv)):
                wtf = consts.tile([P, NE, H], F32, tag=f"wf{name}")
                nc.sync.dma_start(
                    out=wtf, in_=wdram[:, :].rearrange("(c p) h -> p c h", p=P)
                )
                wt = consts.tile([P, NE, H], BF16, tag=f"w{name}")
                nc.vector.tensor_copy(wt, wtf)
                w_tiles[name] = wt

            # rank-1 v-bias operands: ones row [1,128] and bv row [1,64]
            ones_row = consts.tile([1, P], BF16, tag="ones_row")
            nc.vector.memset(ones_row, 1.0)
            bvf = consts.tile([1, H], F32, tag="bvf")
            nc.sync.dma_start(out=bvf, in_=bv[:].rearrange("(o h) -> o h", o=1))
            bvb = consts.tile([1, H], BF16, tag="bvb")
            nc.vector.tensor_copy(bvb, bvf)

            # per-partition bias scalars for q/k projections (q pre-scaled)
            b_tiles = {}
            for name, bdram in (("q", bq), ("k", bk)):
                bt = consts.tile([H, 1], F32, tag=f"b{name}")
                nc.sync.dma_start(
                    out=bt, in_=bdram[:].rearrange("(h o) -> h o", o=1))
                b_tiles[name] = bt
            bqs = consts.tile([H, 1], F32, tag="bqs")
            nc.scalar.mul(out=bqs, in_=b_tiles["q"], mul=SCALE)
            b_tiles["q"] = bqs

            with tc.tile_pool(name="persist", bufs=1) as persist:
                xT = persist.tile([P, NE, T], BF16, tag="xT")
                qT = persist.tile([H, T], BF16, tag="qT")
                kT = persist.tile([H, T], BF16, tag="kT")
                v1 = persist.tile([P, NT, H + 1], BF16, tag="v1")

                for _ in range(reps):
                    _emit_body(nc, tc, x, out, ident, identb, w_tiles,
                               ones_row, bvb, b_tiles,
                               xT, qT, kT, v1)
    nc.compile()
    return nc


def _emit_body(nc, tc, x, out, ident, identb, w_tiles, ones_row, bvb,
               b_tiles, xT, qT, kT, v1):
    gq = int(os.environ.get("KERNEL_GQ", "1024"))
    ng, nb = T // gq, gq // P
    mmw = min(512, gq)
    st_bufs = int(os.environ.get("KERNEL_STB", "2"))
    out_bufs = int(os.environ.get("KERNEL_OUTB", "1"))
    exp_bufs = int(os.environ.get("KERNEL_EXPB", "4"))

    with (
        tc.tile_pool(name="xin", bufs=3) as xin,
        tc.tile_pool(name="ps_set", bufs=2, space="PSUM") as ps_set,
        tc.tile_pool(name="ps_st", bufs=st_bufs, space="PSUM") as ps_st,
        tc.tile_pool(name="ps_out", bufs=out_bufs, space="PSUM") as ps_out,
        tc.tile_pool(name="expp", bufs=exp_bufs) as expp,
        tc.tile_pool(name="expd", bufs=1) as expd,
        tc.tile_pool(name="outsb", bufs=2) as outsb,
        tc.tile_pool(name="stage", bufs=2) as stage,
        tc.tile_pool(name="recp", bufs=4) as recp,
    ):
        ones_col = v1[:, :, H:H + 1]
        nc.vector.memset(ones_col, 1.0)

        outps = {}
        deferred = {}

        def mm1(g, s):
            stp = ps_st.tile([P, gq], F32, tag="st")
            for h2 in range(gq // mmw):
                nc.tensor.matmul(
                    stp[:, h2 * mmw:(h2 + 1) * mmw],
                    kT[:, s * P:(s + 1) * P],
                    qT[:, g * gq + h2 * mmw:g * gq + (h2 + 1) * mmw],
                    start=True,
                    stop=True,
                )
            return stp

        def emit_front(g, s):
            # MM1 + exp only; the exp result is buffered in SBUF and the
            # MM2 is deferred to the post-wave sweep. Fills ACT idle time
            # during the PE-bound wave phase at zero extra PSUM cost.
            stp = mm1(g, s)
            ex = expd.tile([P, gq], BF16, tag=f"exd{g}_{s}",
                           name=f"exd{g}_{s}")
            nc.scalar.activation(
                out=ex, in_=stp, func=mybir.ActivationFunctionType.Exp)
            deferred[(g, s)] = ex

        def emit_attn(its):
            # software pipeline: MM1(i+1) emitted between exp(i) and MM2(i).
            # Iterations whose exp was front-run emit only their MM2.
            # The caller must only pass iterations whose qT/kT/v1 inputs
            # have already been emitted (engine queues are in-order; a
            # lookahead MM1 reaching into a future wave would deadlock).
            live = [it for it in its if it not in deferred]
            li = 0
            stp = None
            if live:
                stp = mm1(*live[0])
                li = 1
            for g, s in its:
                if s == 0:
                    outps[g] = ps_out.tile([H + 1, gq], F32, tag="o",
                                           name=f"outp{g}")
                if (g, s) in deferred:
                    ex = deferred.pop((g, s))
                else:
                    ex = expp.tile([P, gq], BF16, tag="ex")
                    nc.scalar.activation(
                        out=ex, in_=stp,
                        func=mybir.ActivationFunctionType.Exp)
                    if li < len(live):
                        stp = mm1(*live[li])
                        li += 1
                    else:
                        stp = None
                for h2 in range(gq // mmw):
                    nc.tensor.matmul(
                        outps[g][:, h2 * mmw:(h2 + 1) * mmw],
                        v1[:, s, :],
                        ex[:, h2 * mmw:(h2 + 1) * mmw],
                        start=(s == 0),
                        stop=(s == NT - 1),
                    )
                if s == NT - 1:
                    osb = outsb.tile([H + 1, gq], F32, tag="osb",
                                     name=f"osb{g}")
                    nc.vector.tensor_copy(osb, outps.pop(g))
                    _attn_tail(nc, out, ident, osb, stage, recp, ps_set,
                               g, gq, nb)

        mult = mybir.AluOpType.mult
        add = mybir.AluOpType.add
        # in-wave s-offsets whose (g, s) exp is front-run, per group
        front_off = {}
        if ng > 2:
            front_off = {1: (1, 4, 6), 2: (2, 5, 7)}
        elif ng == 2:
            front_off = {1: (1, 3, 5)}
        for w in range(NW):
            # ---- setup for t/s rows [w*1024, (w+1)*1024) ----
            # smaller first chunk so the first transposes start earlier
            chunks = (2, 2, 4) if w == 0 else (4, 4)
            t0 = w * WT
            for sub in chunks:
                xt = xin.tile([P, 4, E], BF16, tag="x")
                nc.gpsimd.dma_start(
                    out=xt[:, 0:sub, :],
                    in_=x[t0 * P:(t0 + sub) * P, :].rearrange(
                        "(i p) e -> p i e", p=P),
                )
                for i_sub in range(sub):
                    i = t0 + i_sub
                    pst = ps_set.tile([P, 512], F32, tag="set")
                    pstb = pst.bitcast(BF16)
                    for c in range(NE):
                        nc.tensor.transpose(
                            pstb[:, c * P:(c + 1) * P],
                            xt[:, i_sub, c * P:(c + 1) * P], identb)
                    nc.vector.tensor_copy(
                        xT[:, :, i * P:(i + 1) * P],
                        pstb[:, 0:NE * P].rearrange("p (c q) -> p c q", q=P))
                t0 += sub
            # q/k projections (k first: MM1 needs it); drain on Pool so the
            # Activation engine does nothing but exp
            for j in range(w * 2, (w + 1) * 2):
                for name, dest, scale in (("k", kT, None), ("q", qT, SCALE)):
                    psp = ps_set.tile([P, 512], F32, tag="set")
                    for c in range(NE):
                        nc.tensor.matmul(
                            psp[0:H, :],
                            w_tiles[name][:, c, :],
                            xT[:, c, j * 512:(j + 1) * 512],
                            start=(c == 0),
                            stop=(c == NE - 1),
                        )
                    if scale is None:
                        nc.vector.tensor_scalar_add(
                            dest[:, j * 512:(j + 1) * 512],
                            psp[0:H, :], b_tiles[name])
                    else:
                        nc.vector.tensor_scalar(
                            out=dest[:, j * 512:(j + 1) * 512],
                            in0=psp[0:H, :],
                            scalar1=scale,
                            scalar2=b_tiles[name],
                            op0=mult,
                            op1=add,
                        )
            # v tiles for this wave, direct [s, h] layout
            for s in range(w * WT, (w + 1) * WT):
                psv = ps_set.tile([P, 512], F32, tag="set")
                for c in range(NE):
                    nc.tensor.matmul(
                        psv[:, 0:H],
                        xT[:, c, s * P:(s + 1) * P],
                        w_tiles["v"][:, c, :],
                        start=(c == 0),
                        stop=False,
                    )
                nc.tensor.matmul(
                    psv[:, 0:H], ones_row, bvb, start=False, stop=True)
                nc.vector.tensor_copy(v1[:, s, 0:H], psv[:, 0:H])
            # ---- attention: group 0, s-tiles of this wave ----
            emit_attn([(0, s) for s in range(w * WT, (w + 1) * WT)])
            # front-run exp for later groups to keep ACT fed (spread so the
            # deferred MM2s interleave evenly with full iterations later)
            if ng > 1:
                for off in front_off.get(1, ()):
                    emit_front(1 % ng, w * WT + off)
                for off in front_off.get(2, ()):
                    emit_front(2 % ng, w * WT + off)

        # ---- remaining groups, one continuous pipelined run ----
        emit_attn([(g, s) for g in range(1, ng) for s in range(NT)])


def _attn_tail(nc, out, ident, osb, stage, recp, ps_tail, g, gq, nb):
    st_t = stage.tile([P, nb, H], F32, tag="stage", name=f"st_t{g}")
    for b in range(nb):
        pst = ps_tail.tile([P, H + 1], F32, tag="set")
        nc.tensor.transpose(
            pst, osb[:, b * P:(b + 1) * P], ident[0:H + 1, 0:H + 1]
        )
        rec = recp.tile([P, 1], F32, tag="rec")
        nc.vector.reciprocal(rec, pst[:, H:H + 1])
        nc.vector.tensor_scalar_mul(st_t[:, b, :], pst[:, 0:H], rec)
        if b % (nb // 2) == nb // 2 - 1:
            # DMA each half as soon as it is ready to hide the tail chain
            h0 = b + 1 - nb // 2
            nc.sync.dma_start(
                out=out[g * gq + h0 * P:g * gq + (b + 1) * P, :].rearrange(
                    "(b p) h -> p b h", p=P),
                in_=st_t[:, h0:b + 1, :],
            )


_NC_CACHE = {}


def _get_nc(key="v2"):
    if key not in _NC_CACHE:
        _NC_CACHE[key] = build_nc()
    return _NC_CACHE[key]


def kernel(x, Wq, bq, Wk, bk, Wv, bv):
    x = np.ascontiguousarray(np.asarray(x, dtype=np.float32))
    in_common = {
        "Wq": np.ascontiguousarray(np.asarray(Wq, np.float32)),
        "Wk": np.ascontiguousarray(np.asarray(Wk, np.float32)),
        "Wv": np.ascontiguousarray(np.asarray(Wv, np.float32)),
        "bq": np.ascontiguousarray(np.asarray(bq, np.float32)),
        "bk": np.ascontiguousarray(np.asarray(bk, np.float32)),
        "bv": np.ascontiguousarray(np.asarray(bv, np.float32)),
    }
    nc = _get_nc()
    in_maps = [dict(in_common, x=x[b]) for b in range(B)]
    res = run_bass_kernel_spmd(nc, in_maps, core_ids=list(range(B)))
    return np.stack([res.results[b]["out"] for b in range(B)], axis=0)


if __name__ == "__main__":
    rng = np.random.default_rng(0)
    xs = rng.standard_normal((B, T, E), dtype=np.float32)
    s = 1.0 / np.sqrt(E)
    mk = lambda *shape: rng.uniform(-s, s, size=shape).astype(np.float32)
    o = kernel(xs, mk(E, H), mk(H), mk(E, H), mk(H), mk(E, H), mk(H))
    print("out", o.shape, o.dtype, float(np.abs(o).max()))


# revision 29
# speedup vs baseline: 6.2334x; 6.2334x over previous
"""Single-head attention (B=8, T=4096, E=768, H=64) on 8 TRN2 NeuronCores.

Sharding: data-parallel over batch B — one batch element per core, Q/K/V
projection weights replicated.

v2 design: wave-interleaved setup + attention emission. Engine queues
execute in program order, so interleaving the emission is what creates
overlap: the Activation engine (the exp bottleneck, ~134us of work at
1024-wide tiles) starts at ~7us instead of after the full setup phase.

Per core:
  consts: identity, W_q/W_k/W_v chunk tiles (bf16), per-partition bias
          scalars (bq pre-scaled by 1/sqrt(H)); ones row + bv row for the
          v-bias rank-1 matmul.

  4 waves over t (1024 rows each); wave w emits:
    - SWDGE cast-load DMAs (f32->bf16) of x t-tiles [Pool queue, which
      does nothing else so DMAs are never blocked behind compute]
    - PE transposes x 128x128 blocks -> xT slices (6 per t-tile batched
      in one PSUM tile, drained by DVE tensor_copy)
    - q/k projections (6 matmuls per 512-chunk, W stationary) drained by
      DVE tensor_scalar (scale*psum+bias, so ACT does nothing but exp)
    - v tiles direct in [s, h] layout: psv[128s, 64h] = sum_c xT_c^T@Wv_c
      + ones x bv (rank-1 bias), DVE-copied into v1 [128, s, 65]
      (col 64 = 1.0 -> MM2 row 64 accumulates the softmax denominator)
    - attention (g=0, s-pairs of this wave) interleaved right away
    - front-run MM1+exp for later groups (g <= w only: group g needs
      wave g's qT), buffering exp pairs in SBUF; their MM2s are deferred
      into the post-wave sweep, spread so drains interleave with full
      iterations. This fills ACT idle time in the PE-bound wave phase.

  attention (software-pipelined over s-tile pairs, q-groups of GQ=1024):
    S^T block [128s, gq] = kT_s.T @ qT on PE (2 x 512-wide matmuls into
    one 2-bank PSUM tile); ONE exp over [128, 1024] on ACT per member
    (wide tiles amortize the ~185ns access-latency cost per ACT
    instruction); out^T [65, gq] += [v|1]_s.T @ exp accumulated in PSUM.
    MM1s of pair i+1 are emitted between the exps and MM2 of pair i.
    tail: PE-transpose out^T blocks (from the ps_set pool - using the
    MM1 pool ring caused pipeline stalls), multiply by reciprocal of the
    denominator, DMA [t, h] half-blocks to DRAM on the SP queue.

  All matmul-facing tensors are bf16 (HW-measured rel err 3.2e-3 vs the
  fp32 reference; gate is 2e-2). fp8 DoubleRow MM2 was tried and reverted:
  quantizing exp/v to fp8e4m3 gives ~2.2e-2 max rel err, over the gate.
"""

import os
import sys

for _p in ("/opt/trn_rl_repo", "/root/.axon_site/_ro/trn_rl_repo"):
    if os.path.isdir(_p) and _p not in sys.path:
        sys.path.insert(0, _p)

import numpy as np

import concourse.bacc as bacc
import concourse.tile as tile
from concourse import mybir
from concourse.bass_utils import run_bass_kernel_spmd
from concourse.masks import make_identity

B, T, E, H = 8, 4096, 768, 64
P = 128
NE = E // P            # 6 e-chunks
NT = T // P            # 32 t/s tiles
SCALE = float(H) ** -0.5

F32 = mybir.dt.float32
BF16 = mybir.dt.bfloat16
F8 = mybir.dt.float8e4

WAVE_T = 1024          # t rows per setup wave
NW = T // WAVE_T       # 4 waves
WT = WAVE_T // P       # 8 t-tiles per wave


def build_nc(attn_dtype=BF16, proj_dtype=BF16, reps=1, rep_scope="all"):
    del attn_dtype, proj_dtype  # bf16-only in v2; args kept for the harness
    nc = bacc.Bacc("TRN2", target_bir_lowering=False, debug=False, num_devices=8)

    x = nc.dram_tensor("x", [T, E], F32, kind="ExternalInput")
    wq = nc.dram_tensor("Wq", [E, H], F32, kind="ExternalInput")
    wk = nc.dram_tensor("Wk", [E, H], F32, kind="ExternalInput")
    wv = nc.dram_tensor("Wv", [E, H], F32, kind="ExternalInput")
    bq = nc.dram_tensor("bq", [H], F32, kind="ExternalInput")
    bk = nc.dram_tensor("bk", [H], F32, kind="ExternalInput")
    bv = nc.dram_tensor("bv", [H], F32, kind="ExternalInput")
    out = nc.dram_tensor("out", [T, H], F32, kind="ExternalOutput")

    with tile.TileContext(nc) as tc:
        with tc.tile_pool(name="consts", bufs=1) as consts:
            ident = consts.tile([P, P], F32)
            make_identity(nc, ident)
            identb = consts.tile([P, P], BF16, tag="identb")
            nc.vector.tensor_copy(identb, ident)

            w_tiles = {}
            for name, wdram in (("q", wq), ("k", wk), ("v", wv)):
                wtf = consts.tile([P, NE, H], F32, tag=f"wf{name}")
                nc.sync.dma_start(
                    out=wtf, in_=wdram[:, :].rearrange("(c p) h -> p c h", p=P)
                )
                wt = consts.tile([P, NE, H], BF16, tag=f"w{name}")
                nc.vector.tensor_copy(wt, wtf)
                w_tiles[name] = wt

            # rank-1 v-bias operands: ones row [1,128] and bv row [1,64]
            ones_row = consts.tile([1, P], BF16, tag="ones_row")
            nc.vector.memset(ones_row, 1.0)
            bvf = consts.tile([1, H], F32, tag="bvf")
            nc.sync.dma_start(out=bvf, in_=bv[:].rearrange("(o h) -> o h", o=1))
            bvb = consts.tile([1, H], BF16, tag="bvb")
            nc.vector.tensor_copy(bvb, bvf)

            # per-partition bias scalars for q/k projections (q pre-scaled)
            b_tiles = {}
            for name, bdram in (("q", bq), ("k", bk)):
                bt = consts.tile([H, 1], F32, tag=f"b{name}")
                nc.sync.dma_start(
                    out=bt, in_=bdram[:].rearrange("(h o) -> h o", o=1))
                b_tiles[name] = bt
            bqs = consts.tile([H, 1], F32, tag="bqs")
            nc.scalar.mul(out=bqs, in_=b_tiles["q"], mul=SCALE)
            b_tiles["q"] = bqs

            with tc.tile_pool(name="persist", bufs=1) as persist:
                xT = persist.tile([P, NE, T], BF16, tag="xT")
                qT = persist.tile([H, T], BF16, tag="qT")
                kT = persist.tile([H, T], BF16, tag="kT")
                v1 = persist.tile([P, NT, H + 1], BF16, tag="v1")

                for _ in range(reps):
                    _emit_body(nc, tc, x, out, ident, identb, w_tiles,
                               ones_row, bvb, b_tiles,
                               xT, qT, kT, v1)
    nc.compile()
    return nc


def _emit_body(nc, tc, x, out, ident, identb, w_tiles, ones_row, bvb,
               b_tiles, xT, qT, kT, v1):
    gq = int(os.environ.get("KERNEL_GQ", "1024"))
    ng, nb = T // gq, gq // P
    mmw = min(512, gq)
    st_bufs = int(os.environ.get("KERNEL_STB", "2"))
    out_bufs = int(os.environ.get("KERNEL_OUTB", "1"))
    exp_bufs = int(os.environ.get("KERNEL_EXPB", "4"))

    with (
        tc.tile_pool(name="xin", bufs=3) as xin,
        tc.tile_pool(name="ps_set", bufs=2, space="PSUM") as ps_set,
        tc.tile_pool(name="ps_st", bufs=st_bufs, space="PSUM") as ps_st,
        tc.tile_pool(name="ps_out", bufs=out_bufs, space="PSUM") as ps_out,
        tc.tile_pool(name="expp", bufs=exp_bufs) as expp,
        tc.tile_pool(name="expd", bufs=1) as expd,
        tc.tile_pool(name="outsb", bufs=2) as outsb,
        tc.tile_pool(name="stage", bufs=2) as stage,
        tc.tile_pool(name="recp", bufs=4) as recp,
    ):
        ones_col = v1[:, :, H:H + 1]
        nc.vector.memset(ones_col, 1.0)

        outps = {}
        deferred = {}

        def mm1(g, s):
            stp = ps_st.tile([P, gq], F32, tag="st")
            for h2 in range(gq // mmw):
                nc.tensor.matmul(
                    stp[:, h2 * mmw:(h2 + 1) * mmw],
                    kT[:, s * P:(s + 1) * P],
                    qT[:, g * gq + h2 * mmw:g * gq + (h2 + 1) * mmw],
                    start=True,
                    stop=True,
                )
            return stp

        NP = NT // 2  # s-tile pairs per group (DoubleRow MM2 unit)
        DR = mybir.MatmulPerfMode.DoubleRow
        EXPF = mybir.ActivationFunctionType.Exp

        def exp_pair(exdst, stps):
            # two exps into the free-dim halves of one [P, 2, gq] fp8 tile
            nc.scalar.activation(out=exdst[:, 0, :], in_=stps[0], func=EXPF)
            nc.scalar.activation(out=exdst[:, 1, :], in_=stps[1], func=EXPF)

        def emit_front(g, sp):
            # MM1s + exps only; the exp pair is buffered in SBUF (fp8) and
            # the MM2 is deferred to the post-wave sweep. Fills ACT idle
            # time during the PE-bound wave phase at zero extra PSUM cost.
            stps = (mm1(g, 2 * sp), mm1(g, 2 * sp + 1))
            ex = expd.tile([P, 2, gq], BF16, tag=f"exd{g}_{sp}",
                           name=f"exd{g}_{sp}")
            exp_pair(ex, stps)
            deferred[(g, sp)] = ex

        def mm2_pair(g, sp, ex):
            for j in range(2):
                for h2 in range(gq // mmw):
                    nc.tensor.matmul(
                        outps[g][:, h2 * mmw:(h2 + 1) * mmw],
                        v1[:, 2 * sp + j, :],
                        ex[:, j, h2 * mmw:(h2 + 1) * mmw],
                        start=(sp == 0 and j == 0),
                        stop=(sp == NP - 1 and j == 1),
                    )

        def emit_attn(its):
            # software pipeline over s-tile PAIRS: MM1s of pair i+1 are
            # emitted between the exps of pair i and MM2 of pair i, so the
            # PE never waits on ACT. Pairs whose exp was front-run emit
            # only their MM2. The caller must only pass pairs whose
            # qT/kT/v1 inputs have already been emitted (engine queues are
            # in-order; a lookahead MM1 into a future wave would deadlock).
            live = [it for it in its if it not in deferred]
            li = 0
            cur = None
            if live:
                g0_, sp0_ = live[0]
                cur = (mm1(g0_, 2 * sp0_), mm1(g0_, 2 * sp0_ + 1))
                li = 1
            for g, sp in its:
                if sp == 0:
                    outps[g] = ps_out.tile([H + 1, gq], F32, tag="o",
                                           name=f"outp{g}")
                if (g, sp) in deferred:
                    ex = deferred.pop((g, sp))
                else:
                    ex = expp.tile([P, 2, gq], BF16, tag="ex")
                    nxt = live[li] if li < len(live) else None
                    nc.scalar.activation(out=ex[:, 0, :], in_=cur[0],
                                         func=EXPF)
                    if nxt:
                        n0 = mm1(nxt[0], 2 * nxt[1])
                    nc.scalar.activation(out=ex[:, 1, :], in_=cur[1],
                                         func=EXPF)
                    if nxt:
                        cur = (n0, mm1(nxt[0], 2 * nxt[1] + 1))
                        li += 1
                    else:
                        cur = None
                mm2_pair(g, sp, ex)
                if sp == NP - 1:
                    osb = outsb.tile([H + 1, gq], F32, tag="osb",
                                     name=f"osb{g}")
                    nc.vector.tensor_copy(osb, outps.pop(g))
                    _attn_tail(nc, out, ident, osb, stage, recp, ps_set,
                               g, gq, nb)

        mult = mybir.AluOpType.mult
        add = mybir.AluOpType.add
        # (g, sp) pairs whose exp is front-run at the end of each wave.
        # Constraints: kT/v1 pair sp needs wave (2*sp)//WT done; qT group g
        # needs wave g done (group g spans qT cols [g*gq, (g+1)*gq)).
        fronts = {}
        if ng == 4 and os.environ.get("KERNEL_NO_FRONT") != "1":
            fronts = {
                1: [(1, 1), (1, 3), (1, 5), (1, 7)],
                2: [(1, 9), (1, 11), (2, 2), (2, 5)],
                3: [(1, 14), (2, 8), (2, 11), (2, 14)],
            }
        for w in range(NW):
            # ---- setup for t/s rows [w*1024, (w+1)*1024) ----
            # smaller first chunk so the first transposes start earlier
            chunks = (2, 2, 4) if w == 0 else (4, 4)
            t0 = w * WT
            for sub in chunks:
                xt = xin.tile([P, 4, E], BF16, tag="x")
                nc.gpsimd.dma_start(
                    out=xt[:, 0:sub, :],
                    in_=x[t0 * P:(t0 + sub) * P, :].rearrange(
                        "(i p) e -> p i e", p=P),
                )
                for i_sub in range(sub):
                    i = t0 + i_sub
                    pst = ps_set.tile([P, 512], F32, tag="set")
                    pstb = pst.bitcast(BF16)
                    for c in range(NE):
                        nc.tensor.transpose(
                            pstb[:, c * P:(c + 1) * P],
                            xt[:, i_sub, c * P:(c + 1) * P], identb)
                    nc.vector.tensor_copy(
                        xT[:, :, i * P:(i + 1) * P],
                        pstb[:, 0:NE * P].rearrange("p (c q) -> p c q", q=P))
                t0 += sub
            # q/k projections (k first: MM1 needs it); drain on Pool so the
            # Activation engine does nothing but exp
            for j in range(w * 2, (w + 1) * 2):
                for name, dest, scale in (("k", kT, None), ("q", qT, SCALE)):
                    psp = ps_set.tile([P, 512], F32, tag="set")
                    for c in range(NE):
                        nc.tensor.matmul(
                            psp[0:H, :],
                            w_tiles[name][:, c, :],
                            xT[:, c, j * 512:(j + 1) * 512],
                            start=(c == 0),
                            stop=(c == NE - 1),
                        )
                    if scale is None:
                        nc.vector.tensor_scalar_add(
                            dest[:, j * 512:(j + 1) * 512],
                            psp[0:H, :], b_tiles[name])
                    else:
                        nc.vector.tensor_scalar(
                            out=dest[:, j * 512:(j + 1) * 512],
                            in0=psp[0:H, :],
                            scalar1=scale,
                            scalar2=b_tiles[name],
                            op0=mult,
                            op1=add,
                        )
            # v tiles for this wave, direct [s, h] layout
            for s in range(w * WT, (w + 1) * WT):
                psv = ps_set.tile([P, 512], F32, tag="set")
                for c in range(NE):
                    nc.tensor.matmul(
                        psv[:, 0:H],
                        xT[:, c, s * P:(s + 1) * P],
                        w_tiles["v"][:, c, :],
                        start=(c == 0),
                        stop=False,
                    )
                nc.tensor.matmul(
                    psv[:, 0:H], ones_row, bvb, start=False, stop=True)
                nc.vector.tensor_copy(v1[:, s, 0:H], psv[:, 0:H])
            # ---- attention: group 0, s-tile pairs of this wave ----
            # (the last wave's pairs merge into the post-phase run below so
            # the software pipeline never drains at the transition)
            if w < NW - 1:
                emit_attn([(0, sp)
                           for sp in range(w * WT // 2, (w + 1) * WT // 2)])
                for g, sp in fronts.get(w, ()):
                    emit_front(g, sp)

        # ---- last wave's g0 pairs + remaining groups, one pipelined run ----
        emit_attn([(0, sp) for sp in range((NW - 1) * WT // 2, NT // 2)] +
                  [(g, sp) for g in range(1, ng) for sp in range(NT // 2)])


def _attn_tail(nc, out, ident, osb, stage, recp, ps_tail, g, gq, nb):
    st_t = stage.tile([P, nb, H], F32, tag="stage", name=f"st_t{g}")
    for b in range(nb):
        pst = ps_tail.tile([P, H + 1], F32, tag="set")
        nc.tensor.transpose(
            pst, osb[:, b * P:(b + 1) * P], ident[0:H + 1, 0:H + 1]
        )
        rec = recp.tile([P, 1], F32, tag="rec")
        nc.vector.reciprocal(rec, pst[:, H:H + 1])
        nc.vector.tensor_scalar_mul(st_t[:, b, :], pst[:, 0:H], rec)
        if b % (nb // 2) == nb // 2 - 1:
            # DMA each half as soon as it is ready to hide the tail chain
            h0 = b + 1 - nb // 2
            nc.sync.dma_start(
                out=out[g * gq + h0 * P:g * gq + (b + 1) * P, :].rearrange(
                    "(b p) h -> p b h", p=P),
                in_=st_t[:, h0:b + 1, :],
            )


_NC_CACHE = {}


def _get_nc(key="v2"):
    if key not in _NC_CACHE:
        _NC_CACHE[key] = build_nc()
    return _NC_CACHE[key]


def kernel(x, Wq, bq, Wk, bk, Wv, bv):
    x = np.ascontiguousarray(np.asarray(x, dtype=np.float32))
    in_common = {
        "Wq": np.ascontiguousarray(np.asarray(Wq, np.float32)),
        "Wk": np.ascontiguousarray(np.asarray(Wk, np.float32)),
        "Wv": np.ascontiguousarray(np.asarray(Wv, np.float32)),
        "bq": np.ascontiguousarray(np.asarray(bq, np.float32)),
        "bk": np.ascontiguousarray(np.asarray(bk, np.float32)),
        "bv": np.ascontiguousarray(np.asarray(bv, np.float32)),
    }
    nc = _get_nc()
    in_maps = [dict(in_common, x=x[b]) for b in range(B)]
    res = run_bass_kernel_spmd(nc, in_maps, core_ids=list(range(B)))
    return np.stack([res.results[b]["out"] for b in range(B)], axis=0)


if __name__ == "__main__":
    rng = np.random.default_rng(0)
    xs = rng.standard_normal((B, T, E), dtype=np.float32)
    s = 1.0 / np.sqrt(E)
    mk = lambda *shape: rng.uniform(-s, s, size=shape).astype(np.float32)
    o = kernel(xs, mk(E, H), mk(H), mk(E, H), mk(H), mk(E, H), mk(H))
    print("out", o.shape, o.dtype, float(np.abs(o).max()))


# revision 34
# speedup vs baseline: 7.1899x; 1.1534x over previous
"""Single-head attention (B=8, T=4096, E=768, H=64) on 8 TRN2 NeuronCores.

Sharding: data-parallel over batch B — one batch element per core, Q/K/V
projection weights replicated.

v2 design: wave-interleaved setup + attention emission. Engine queues
execute in program order, so interleaving the emission is what creates
overlap: the Activation engine (the exp bottleneck, ~134us of work at
1024-wide tiles) starts at ~7us instead of after the full setup phase.

Per core:
  consts: identity, W_q/W_k/W_v chunk tiles (bf16), per-partition bias
          scalars (bq pre-scaled by 1/sqrt(H)); ones row + bv row for the
          v-bias rank-1 matmul.

  4 waves over t (1024 rows each); wave w emits:
    - SWDGE cast-load DMAs (f32->bf16) of x t-tiles [Pool queue, which
      does nothing else so DMAs are never blocked behind compute]
    - PE transposes x 128x128 blocks -> xT slices (6 per t-tile batched
      in one PSUM tile, drained by DVE tensor_copy)
    - q/k projections (6 matmuls per 512-chunk, W stationary) drained by
      DVE tensor_scalar (scale*psum+bias, so ACT does nothing but exp)
    - v tiles direct in [s, h] layout: psv[128s, 64h] = sum_c xT_c^T@Wv_c
      + ones x bv (rank-1 bias), DVE-copied into v1 [128, s, 65]
      (col 64 = 1.0 -> MM2 row 64 accumulates the softmax denominator)
    - attention (g=0, s-pairs of this wave) interleaved right away
    - front-run MM1+exp for later groups (g <= w only: group g needs
      wave g's qT), buffering exp pairs in SBUF; their MM2s are deferred
      into the post-wave sweep, spread so drains interleave with full
      iterations. This fills ACT idle time in the PE-bound wave phase.

  attention (software-pipelined over s-tile pairs, q-groups of GQ=1024):
    S^T block [128s, gq] = kT_s.T @ qT on PE (2 x 512-wide matmuls into
    one 2-bank PSUM tile); ONE exp over [128, 1024] on ACT per member
    (wide tiles amortize the ~185ns access-latency cost per ACT
    instruction); out^T [65, gq] += [v|1]_s.T @ exp accumulated in PSUM.
    MM1s of pair i+1 are emitted between the exps and MM2 of pair i.
    tail: PE-transpose out^T blocks (from the ps_set pool - using the
    MM1 pool ring caused pipeline stalls), multiply by reciprocal of the
    denominator, DMA [t, h] half-blocks to DRAM on the SP queue.

  All matmul-facing tensors are bf16 (HW-measured rel err 3.2e-3 vs the
  fp32 reference; gate is 2e-2). fp8 DoubleRow MM2 was tried and reverted:
  quantizing exp/v to fp8e4m3 gives ~2.2e-2 max rel err, over the gate.
"""

import os
import sys

for _p in ("/opt/trn_rl_repo", "/root/.axon_site/_ro/trn_rl_repo"):
    if os.path.isdir(_p) and _p not in sys.path:
        sys.path.insert(0, _p)

import numpy as np

import concourse.bacc as bacc
import concourse.tile as tile
from concourse import mybir
from concourse.bass_utils import run_bass_kernel_spmd
from concourse.masks import make_identity

B, T, E, H = 8, 4096, 768, 64
P = 128
NE = E // P            # 6 e-chunks
NT = T // P            # 32 t/s tiles
SCALE = float(H) ** -0.5

F32 = mybir.dt.float32
BF16 = mybir.dt.bfloat16
F8 = mybir.dt.float8e4

WAVE_T = 1024          # t rows per setup wave
NW = T // WAVE_T       # 4 waves
WT = WAVE_T // P       # 8 t-tiles per wave


def build_nc(attn_dtype=BF16, proj_dtype=BF16, reps=1, rep_scope="all"):
    del attn_dtype, proj_dtype  # bf16-only in v2; args kept for the harness
    nc = bacc.Bacc("TRN2", target_bir_lowering=False, debug=False, num_devices=8)

    x = nc.dram_tensor("x", [T, E], F32, kind="ExternalInput")
    wq = nc.dram_tensor("Wq", [E, H], F32, kind="ExternalInput")
    wk = nc.dram_tensor("Wk", [E, H], F32, kind="ExternalInput")
    wv = nc.dram_tensor("Wv", [E, H], F32, kind="ExternalInput")
    bq = nc.dram_tensor("bq", [H], F32, kind="ExternalInput")
    bk = nc.dram_tensor("bk", [H], F32, kind="ExternalInput")
    bv = nc.dram_tensor("bv", [H], F32, kind="ExternalInput")
    out = nc.dram_tensor("out", [T, H], F32, kind="ExternalOutput")

    with tile.TileContext(nc) as tc:
        with tc.tile_pool(name="consts", bufs=1) as consts:
            ident = consts.tile([P, P], F32)
            make_identity(nc, ident)
            identb = consts.tile([P, P], BF16, tag="identb")
            nc.vector.tensor_copy(identb, ident)

            w_tiles = {}
            for name, wdram in (("q", wq), ("k", wk), ("v", wv)):
                wtf = consts.tile([P, NE, H], F32, tag=f"wf{name}")
                nc.sync.dma_start(
                    out=wtf, in_=wdram[:, :].rearrange("(c p) h -> p c h", p=P)
                )
                wt = consts.tile([P, NE, H], BF16, tag=f"w{name}")
                nc.vector.tensor_copy(wt, wtf)
                w_tiles[name] = wt

            # per-partition bias scalars for q/k/v projections (q pre-scaled)
            b_tiles = {}
            for name, bdram in (("q", bq), ("k", bk), ("v", bv)):
                bt = consts.tile([H, 1], F32, tag=f"b{name}")
                nc.sync.dma_start(
                    out=bt, in_=bdram[:].rearrange("(h o) -> h o", o=1))
                b_tiles[name] = bt
            bqs = consts.tile([H, 1], F32, tag="bqs")
            nc.scalar.mul(out=bqs, in_=b_tiles["q"], mul=SCALE)
            b_tiles["q"] = bqs

            with tc.tile_pool(name="persist", bufs=1) as persist:
                xT = persist.tile([P, NE, T], BF16, tag="xT")
                qT = persist.tile([H, T], BF16, tag="qT")
                kT = persist.tile([H, T], BF16, tag="kT")
                vT = persist.tile([H, T], BF16, tag="vT")
                v1 = persist.tile([P, NT, H + 1], BF16, tag="v1")

                for _ in range(reps):
                    _emit_body(nc, tc, x, out, ident, identb, w_tiles,
                               b_tiles, xT, qT, kT, vT, v1)
    nc.compile()
    return nc


def _emit_body(nc, tc, x, out, ident, identb, w_tiles,
               b_tiles, xT, qT, kT, vT, v1):
    gq = int(os.environ.get("KERNEL_GQ", "1024"))
    ng, nb = T // gq, gq // P
    mmw = min(512, gq)
    st_bufs = int(os.environ.get("KERNEL_STB", "2"))
    out_bufs = int(os.environ.get("KERNEL_OUTB", "1"))
    exp_bufs = int(os.environ.get("KERNEL_EXPB", "4"))

    with (
        tc.tile_pool(name="xin", bufs=3) as xin,
        tc.tile_pool(name="ps_set", bufs=2, space="PSUM") as ps_set,
        tc.tile_pool(name="ps_st", bufs=st_bufs, space="PSUM") as ps_st,
        tc.tile_pool(name="ps_out", bufs=out_bufs, space="PSUM") as ps_out,
        tc.tile_pool(name="expp", bufs=exp_bufs) as expp,
        tc.tile_pool(name="expd", bufs=1) as expd,
        tc.tile_pool(name="outsb", bufs=2) as outsb,
        tc.tile_pool(name="stage", bufs=2) as stage,
        tc.tile_pool(name="recp", bufs=4) as recp,
    ):
        ones_col = v1[:, :, H:H + 1]
        nc.vector.memset(ones_col, 1.0)

        outps = {}
        deferred = {}

        def mm1(g, s):
            stp = ps_st.tile([P, gq], F32, tag="st")
            for h2 in range(gq // mmw):
                nc.tensor.matmul(
                    stp[:, h2 * mmw:(h2 + 1) * mmw],
                    kT[:, s * P:(s + 1) * P],
                    qT[:, g * gq + h2 * mmw:g * gq + (h2 + 1) * mmw],
                    start=True,
                    stop=True,
                )
            return stp

        NP = NT // 2  # s-tile pairs per group (DoubleRow MM2 unit)
        DR = mybir.MatmulPerfMode.DoubleRow
        EXPF = mybir.ActivationFunctionType.Exp

        def exp_pair(exdst, stps):
            # two exps into the free-dim halves of one [P, 2, gq] fp8 tile
            nc.scalar.activation(out=exdst[:, 0, :], in_=stps[0], func=EXPF)
            nc.scalar.activation(out=exdst[:, 1, :], in_=stps[1], func=EXPF)

        def emit_front(g, sp):
            # MM1s + exps only; the exp pair is buffered in SBUF (fp8) and
            # the MM2 is deferred to the post-wave sweep. Fills ACT idle
            # time during the PE-bound wave phase at zero extra PSUM cost.
            stps = (mm1(g, 2 * sp), mm1(g, 2 * sp + 1))
            ex = expd.tile([P, 2, gq], BF16, tag=f"exd{g}_{sp}",
                           name=f"exd{g}_{sp}")
            exp_pair(ex, stps)
            deferred[(g, sp)] = ex

        def mm2_pair(g, sp, ex):
            for j in range(2):
                for h2 in range(gq // mmw):
                    nc.tensor.matmul(
                        outps[g][:, h2 * mmw:(h2 + 1) * mmw],
                        v1[:, 2 * sp + j, :],
                        ex[:, j, h2 * mmw:(h2 + 1) * mmw],
                        start=(sp == 0 and j == 0),
                        stop=(sp == NP - 1 and j == 1),
                    )

        def emit_attn(its):
            # software pipeline over s-tile PAIRS: MM1s of pair i+1 are
            # emitted between the exps of pair i and MM2 of pair i, so the
            # PE never waits on ACT. Pairs whose exp was front-run emit
            # only their MM2. The caller must only pass pairs whose
            # qT/kT/v1 inputs have already been emitted (engine queues are
            # in-order; a lookahead MM1 into a future wave would deadlock).
            live = [it for it in its if it not in deferred]
            li = 0
            cur = None
            if live:
                g0_, sp0_ = live[0]
                cur = (mm1(g0_, 2 * sp0_), mm1(g0_, 2 * sp0_ + 1))
                li = 1
            for g, sp in its:
                if sp == 0:
                    outps[g] = ps_out.tile([H + 1, gq], F32, tag="o",
                                           name=f"outp{g}")
                if (g, sp) in deferred:
                    ex = deferred.pop((g, sp))
                else:
                    ex = expp.tile([P, 2, gq], BF16, tag="ex")
                    nxt = live[li] if li < len(live) else None
                    nc.scalar.activation(out=ex[:, 0, :], in_=cur[0],
                                         func=EXPF)
                    if nxt:
                        n0 = mm1(nxt[0], 2 * nxt[1])
                    nc.scalar.activation(out=ex[:, 1, :], in_=cur[1],
                                         func=EXPF)
                    if nxt:
                        cur = (n0, mm1(nxt[0], 2 * nxt[1] + 1))
                        li += 1
                    else:
                        cur = None
                mm2_pair(g, sp, ex)
                if sp == NP - 1:
                    osb = outsb.tile([H + 1, gq], F32, tag="osb",
                                     name=f"osb{g}")
                    nc.vector.tensor_copy(osb, outps.pop(g))
                    _attn_tail(nc, out, ident, osb, stage, recp, ps_set,
                               g, gq, nb)

        mult = mybir.AluOpType.mult
        add = mybir.AluOpType.add
        # (g, sp) pairs whose exp is front-run at the end of each wave.
        # Constraints: kT/v1 pair sp needs wave (2*sp)//WT done; qT group g
        # needs wave g done (group g spans qT cols [g*gq, (g+1)*gq)).
        fronts = {}
        if ng == 4 and os.environ.get("KERNEL_NO_FRONT") != "1":
            fronts = {
                1: [(1, 1), (1, 3), (1, 5), (1, 7)],
                2: [(1, 9), (1, 11), (2, 2), (2, 5)],
                3: [(1, 14), (2, 8), (2, 11), (2, 14)],
            }
        for w in range(NW):
            # ---- setup for t/s rows [w*1024, (w+1)*1024) ----
            # smaller first chunk so the first transposes start earlier
            chunks = (2, 2, 4) if w == 0 else (4, 4)
            t0 = w * WT
            for sub in chunks:
                xt = xin.tile([P, 4, E], BF16, tag="x")
                nc.gpsimd.dma_start(
                    out=xt[:, 0:sub, :],
                    in_=x[t0 * P:(t0 + sub) * P, :].rearrange(
                        "(i p) e -> p i e", p=P),
                )
                for i_sub in range(sub):
                    i = t0 + i_sub
                    pst = ps_set.tile([P, 512], F32, tag="set")
                    pstb = pst.bitcast(BF16)
                    for c in range(NE):
                        nc.tensor.transpose(
                            pstb[:, c * P:(c + 1) * P],
                            xt[:, i_sub, c * P:(c + 1) * P], identb)
                    nc.vector.tensor_copy(
                        xT[:, :, i * P:(i + 1) * P],
                        pstb[:, 0:NE * P].rearrange("p (c q) -> p c q", q=P))
                t0 += sub
            # q/k/v projections (k first: MM1 needs it); drained on DVE so
            # the Activation engine does nothing but exp. W stays stationary
            # across 512 moving columns (ldweights amortization on HW).
            for j in range(w * 2, (w + 1) * 2):
                for name, dest, scale in (("k", kT, None), ("q", qT, SCALE),
                                          ("v", vT, None)):
                    psp = ps_set.tile([P, 512], F32, tag="set")
                    for c in range(NE):
                        nc.tensor.matmul(
                            psp[0:H, :],
                            w_tiles[name][:, c, :],
                            xT[:, c, j * 512:(j + 1) * 512],
                            start=(c == 0),
                            stop=(c == NE - 1),
                        )
                    if scale is None:
                        nc.vector.tensor_scalar_add(
                            dest[:, j * 512:(j + 1) * 512],
                            psp[0:H, :], b_tiles[name])
                    else:
                        nc.vector.tensor_scalar(
                            out=dest[:, j * 512:(j + 1) * 512],
                            in0=psp[0:H, :],
                            scalar1=scale,
                            scalar2=b_tiles[name],
                            op0=mult,
                            op1=add,
                        )
            # v tiles for this wave: PE-transpose vT -> v1 [s, h] layout
            for s in range(w * WT, (w + 1) * WT):
                psv = ps_set.tile([P, 512], F32, tag="set",
                                  name="psv").bitcast(BF16)
                nc.tensor.transpose(
                    psv[:, 0:H], vT[:, s * P:(s + 1) * P], identb[0:H, 0:H])
                nc.vector.tensor_copy(v1[:, s, 0:H], psv[:, 0:H])
            # ---- attention: group 0, s-tile pairs of this wave ----
            # (the last wave's pairs merge into the post-phase run below so
            # the software pipeline never drains at the transition)
            if w < NW - 1:
                emit_attn([(0, sp)
                           for sp in range(w * WT // 2, (w + 1) * WT // 2)])
                for g, sp in fronts.get(w, ()):
                    emit_front(g, sp)

        # ---- last wave's g0 pairs + remaining groups, one pipelined run ----
        emit_attn([(0, sp) for sp in range((NW - 1) * WT // 2, NT // 2)] +
                  [(g, sp) for g in range(1, ng) for sp in range(NT // 2)])


def _attn_tail(nc, out, ident, osb, stage, recp, ps_tail, g, gq, nb):
    st_t = stage.tile([P, nb, H], F32, tag="stage", name=f"st_t{g}")
    for b in range(nb):
        pst = ps_tail.tile([P, H + 1], F32, tag="set")
        nc.tensor.transpose(
            pst, osb[:, b * P:(b + 1) * P], ident[0:H + 1, 0:H + 1]
        )
        rec = recp.tile([P, 1], F32, tag="rec")
        nc.vector.reciprocal(rec, pst[:, H:H + 1])
        nc.vector.tensor_scalar_mul(st_t[:, b, :], pst[:, 0:H], rec)
        if b % (nb // 2) == nb // 2 - 1:
            # DMA each half as soon as it is ready to hide the tail chain
            h0 = b + 1 - nb // 2
            nc.sync.dma_start(
                out=out[g * gq + h0 * P:g * gq + (b + 1) * P, :].rearrange(
                    "(b p) h -> p b h", p=P),
                in_=st_t[:, h0:b + 1, :],
            )


_NC_CACHE = {}


def _get_nc(key="v2"):
    if key not in _NC_CACHE:
        _NC_CACHE[key] = build_nc()
    return _NC_CACHE[key]


def kernel(x, Wq, bq, Wk, bk, Wv, bv):
    x = np.ascontiguousarray(np.asarray(x, dtype=np.float32))
    in_common = {
        "Wq": np.ascontiguousarray(np.asarray(Wq, np.float32)),
        "Wk": np.ascontiguousarray(np.asarray(Wk, np.float32)),
        "Wv": np.ascontiguousarray(np.asarray(Wv, np.float32)),
        "bq": np.ascontiguousarray(np.asarray(bq, np.float32)),
        "bk": np.ascontiguousarray(np.asarray(bk, np.float32)),
        "bv": np.ascontiguousarray(np.asarray(bv, np.float32)),
    }
    nc = _get_nc()
    in_maps = [dict(in_common, x=x[b]) for b in range(B)]
    res = run_bass_kernel_spmd(nc, in_maps, core_ids=list(range(B)))
    return np.stack([res.results[b]["out"] for b in range(B)], axis=0)


if __name__ == "__main__":
    rng = np.random.default_rng(0)
    xs = rng.standard_normal((B, T, E), dtype=np.float32)
    s = 1.0 / np.sqrt(E)
    mk = lambda *shape: rng.uniform(-s, s, size=shape).astype(np.float32)
    o = kernel(xs, mk(E, H), mk(H), mk(E, H), mk(H), mk(E, H), mk(H))
    print("out", o.shape, o.dtype, float(np.abs(o).max()))
